# revision 2
# baseline (speedup 1.0000x reference)
"""CityGRU (embedding-lookup GRU cell + vocab projection) on 8 TRN2 NeuronCores.

Reference computation (B=1024, H=1024, vocab=50000):
    x_gates = W_ih.T[idx] + b_ih                  # embedding gather  [B, 3H]
    h_gates = h @ W_hh.T + b_hh                   # [B, 3H]
    r = sigmoid(xr + hr); z = sigmoid(xz + hz); n = tanh(xn + r * hn)
    h_new = (1 - z) * n + z * h
    logit = tanh(h_new @ W_out.T + b_out)         # [B, 50000]

Sharding (8 cores, tensor-parallel over the vocab):
  * core k owns vocab rows [k*6250, (k+1)*6250) of W_ih / W_out / b_out.
  * the batch is host-permuted so that core k's GRU rows are exactly the
    indices its W_ih shard owns; the gather happens on-device via
    indirect DMA from the core's local (bias-folded) embedding table.
  * each core computes the GRU for its ~128 rows (padded to a uniform
    capacity), transposes h_new_seg, AllGathers the transposed segments,
    compacts them into the full h_newT [H, B], then computes its vocab
    shard of the logit matmul + tanh.
  * matmuls run in float32r (full-rate fp32, ~1e-4 rel err).

Host does only sharding/layout work: permutation, padding, weight
transposes, bias folding, and final unpermute/concat.
"""

import numpy as np

import concourse.bass as bass
import concourse.mybir as mybir
import concourse.tile as tile
from concourse import bacc
from concourse import bass_utils
from concourse.masks import make_identity

N_CORES = 8
B = 1024
H = 1024
G = 3 * H          # 3072 gate width
V = 50000
VS = V // N_CORES  # 6250 vocab rows per core
NV = 49            # vocab tiles of 128 per core (6272 padded)
VP = NV * 128      # 6272
KT = H // 128      # 8 hidden k-tiles
KA = KT + 1        # 9 augmented k-tiles (bias row)

# Set by an external harness to request NTFF tracing (requires the
# antenv.axon_hooks profiling hook to be installed by that harness).
TRACE = False
LAST_RESULT = None  # BassKernelResults of the most recent run (for harnesses)

_BUILD_CACHE = {}


def _build(c_tiles, cnts, offs):
    """Build + compile the SPMD Bass program.

    c_tiles: number of 128-row tiles of per-core GRU batch capacity.
    cnts/offs: per-rank segment sizes/offsets in the permuted batch
    (baked into the compaction DMA addressing; identical on all cores).
    """
    dt = mybir.dt
    C = 128 * c_tiles

    nc = bacc.Bacc("TRN2", target_bir_lowering=False, debug=False,
                   num_devices=N_CORES)

    # ---- per-core inputs ----
    table = nc.dram_tensor("table", [VS, G], dt.float32, kind="ExternalInput")
    locidx = nc.dram_tensor("locidx", [c_tiles, 128, 1], dt.int32,
                            kind="ExternalInput")
    hsegT = nc.dram_tensor("hsegT", [KA * 128, C], dt.float32r,
                           kind="ExternalInput")
    hseg = nc.dram_tensor("hseg", [C, H], dt.float32, kind="ExternalInput")
    whhT = nc.dram_tensor("whhT", [KA * 128, G], dt.float32r,
                          kind="ExternalInput")
    wout = nc.dram_tensor("wout", [NV, 128, H], dt.float32r,
                          kind="ExternalInput")
    bout = nc.dram_tensor("bout", [128, NV], dt.float32, kind="ExternalInput")

    # ---- per-core outputs ----
    logit_sh = nc.dram_tensor("logit_sh", [VP, B], dt.float32,
                              kind="ExternalOutput")
    hseg_out = nc.dram_tensor("hseg_out", [C, H], dt.float32,
                              kind="ExternalOutput")

    SIG = mybir.ActivationFunctionType.Sigmoid
    TANH = mybir.ActivationFunctionType.Tanh

    with tile.TileContext(nc) as tc:
        with (
            tc.tile_pool(name="pers", bufs=1) as pers,
            tc.tile_pool(name="dram", bufs=1, space="DRAM") as dram,
        ):
            ident = pers.tile([128, 128], dt.float32, name="ident", tag="ident")
            make_identity(nc, ident[:])

            bout_t = pers.tile([128, NV], dt.float32, name="bout", tag="bout")
            nc.sync.dma_start(bout_t[:], bout[:])

            # h_newT (full, [H, B] as 8 tiles) -- filled after the AllGather
            hT_full = [pers.tile([128, B], dt.float32r, name=f"hTf{k}", tag=f"hTf{k}")
                       for k in range(KT)]

            ag_in = dram.tile([KT * 128, C], dt.float32)
            ag_out = dram.tile([N_CORES * KT * 128, C], dt.float32)

            # ---------------- Phase A: gather + GRU on the local segment ----
            x_seg = []
            hseg_t = []
            hnew_t = []
            for mt in range(c_tiles):
                idx_t = pers.tile([128, 1], dt.int32, name=f"idx{mt}", tag=f"idx{mt}")
                nc.sync.dma_start(idx_t[:], locidx[mt])
                xs = pers.tile([128, G], dt.float32, name=f"xseg{mt}", tag=f"xseg{mt}")
                nc.gpsimd.indirect_dma_start(
                    out=xs[:],
                    out_offset=None,
                    in_=table[:],
                    in_offset=bass.IndirectOffsetOnAxis(ap=idx_t[:, :1], axis=0),
                )
                x_seg.append(xs)
                hs = pers.tile([128, H], dt.float32, name=f"hseg{mt}", tag=f"hseg{mt}")
                nc.sync.dma_start(hs[:], hseg[mt * 128:(mt + 1) * 128, :])
                hseg_t.append(hs)
                hnew_t.append(pers.tile([128, H], dt.float32, name=f"hnew{mt}", tag=f"hnew{mt}"))

            hsegT_t = []
            for k in range(KA):
                t = pers.tile([128, C], dt.float32r, name=f"hsT{k}", tag=f"hsT{k}")
                nc.sync.dma_start(t[:], hsegT[k * 128:(k + 1) * 128, :])
                hsegT_t.append(t)

            with (
                tc.tile_pool(name="whhp", bufs=12) as whhp,
                tc.tile_pool(name="psA", bufs=6, space="PSUM") as psA,
                tc.tile_pool(name="gate", bufs=1) as gate,
            ):
                r_t = [gate.tile([128, H], dt.float32, name=f"r{mt}", tag=f"r{mt}")
                       for mt in range(c_tiles)]
                z_t = [gate.tile([128, H], dt.float32, name=f"z{mt}", tag=f"z{mt}")
                       for mt in range(c_tiles)]
                n_t = [gate.tile([128, H], dt.float32, name=f"n{mt}", tag=f"n{mt}")
                       for mt in range(c_tiles)]
                tmp_r = [gate.tile([128, H], dt.float32, name=f"tr{mt}", tag=f"tr{mt}")
                         for mt in range(c_tiles)]
                tmp_z = [gate.tile([128, H], dt.float32, name=f"tz{mt}", tag=f"tz{mt}")
                         for mt in range(c_tiles)]
                tmp_n = [gate.tile([128, H], dt.float32, name=f"tn{mt}", tag=f"tn{mt}")
                         for mt in range(c_tiles)]
                tmp_h = [gate.tile([128, H], dt.float32, name=f"th{mt}", tag=f"th{mt}")
                         for mt in range(c_tiles)]

                # h_gates = hseg @ W_hh_aug.T in 512-wide gate chunks.
                ps = {}
                for n in range(6):
                    w_nk = []
                    for k in range(KA):
                        w = whhp.tile([128, 512], dt.float32r, name="whh", tag="whh")
                        nc.sync.dma_start(
                            w[:], whhT[k * 128:(k + 1) * 128,
                                       n * 512:(n + 1) * 512])
                        w_nk.append(w)
                    for mt in range(c_tiles):
                        p = psA.tile([128, 512], dt.float32, name="psA", tag="psA")
                        for k in range(KA):
                            nc.tensor.matmul(
                                p[:],
                                hsegT_t[k][:, mt * 128:(mt + 1) * 128],
                                w_nk[k][:],
                                start=(k == 0),
                                stop=(k == KA - 1),
                            )
                        ps[(mt, n)] = p

                    # consume psum pairs as soon as a gate's two chunks exist
                    if n == 1:  # r gate: chunks 0,1
                        for mt in range(c_tiles):
                            for j in (0, 1):
                                nc.vector.tensor_add(
                                    tmp_r[mt][:, j * 512:(j + 1) * 512],
                                    ps[(mt, j)][:],
                                    x_seg[mt][:, j * 512:(j + 1) * 512])
                            nc.scalar.activation(r_t[mt][:], tmp_r[mt][:], SIG)
                    if n == 3:  # z gate: chunks 2,3
                        for mt in range(c_tiles):
                            for j in (2, 3):
                                nc.vector.tensor_add(
                                    tmp_z[mt][:, (j - 2) * 512:(j - 1) * 512],
                                    ps[(mt, j)][:],
                                    x_seg[mt][:, j * 512:(j + 1) * 512])
                            nc.scalar.activation(z_t[mt][:], tmp_z[mt][:], SIG)
                    if n == 5:  # n gate: chunks 4,5 (x + r * h_gates)
                        for mt in range(c_tiles):
                            for j in (4, 5):
                                s = slice((j - 4) * 512, (j - 3) * 512)
                                nc.vector.tensor_mul(
                                    tmp_n[mt][:, s], r_t[mt][:, s],
                                    ps[(mt, j)][:])
                                nc.vector.tensor_add(
                                    tmp_n[mt][:, s], tmp_n[mt][:, s],
                                    x_seg[mt][:, j * 512:(j + 1) * 512])
                            nc.scalar.activation(n_t[mt][:], tmp_n[mt][:], TANH)

                # h_new = n + z * (h - n)
                for mt in range(c_tiles):
                    nc.vector.tensor_sub(tmp_h[mt][:], hseg_t[mt][:],
                                         n_t[mt][:])
                    nc.vector.tensor_mul(tmp_h[mt][:], z_t[mt][:],
                                         tmp_h[mt][:])
                    nc.vector.tensor_add(hnew_t[mt][:], n_t[mt][:],
                                         tmp_h[mt][:])
                    nc.sync.dma_start(hseg_out[mt * 128:(mt + 1) * 128, :],
                                      hnew_t[mt][:])

                # transpose h_new_seg -> [H, C] tiles, ship to the AllGather
                hnT = [gate.tile([128, C], dt.float32, name=f"hnT{k}", tag=f"hnT{k}")
                       for k in range(KT)]
                with tc.tile_pool(name="psT", bufs=2, space="PSUM") as psT:
                    for mt in range(c_tiles):
                        for k in range(KT):
                            pt = psT.tile([128, 128], dt.float32, name="psT", tag="psT")
                            nc.tensor.transpose(
                                pt[:],
                                hnew_t[mt][:, k * 128:(k + 1) * 128],
                                ident[:])
                            nc.vector.tensor_copy(
                                hnT[k][:, mt * 128:(mt + 1) * 128], pt[:])
                for k in range(KT):
                    nc.sync.dma_start(ag_in[k * 128:(k + 1) * 128, :],
                                      hnT[k][:])

                # ---------------- Phase B: AllGather + compaction ----------
                nc.gpsimd.collective_compute(
                    "AllGather",
                    mybir.AluOpType.bypass,
                    ins=[ag_in.opt()],
                    outs=[ag_out.opt()],
                    replica_groups=[list(range(N_CORES))],
                )
                for r in range(N_CORES):
                    cnt = int(cnts[r])
                    if cnt == 0:
                        continue
                    off = int(offs[r])
                    for k in range(KT):
                        row0 = r * KT * 128 + k * 128
                        nc.sync.dma_start(
                            hT_full[k][:, off:off + cnt],
                            ag_out[row0:row0 + 128, 0:cnt].bitcast(
                                dt.float32r))

            # ---------------- Phase C: logit = tanh(Wout_sh @ h_newT + b) --
            with (
                tc.tile_pool(name="woutp", bufs=3) as woutp,
                tc.tile_pool(name="lop", bufs=3) as lop,
                tc.tile_pool(name="psC", bufs=3, space="PSUM") as psC,
            ):
                for v in range(NV):
                    wt = woutp.tile([128, H], dt.float32r, name="wout", tag="wout")
                    nc.sync.dma_start(wt[:], wout[v])
                    p = psC.tile([128, B], dt.float32, name="psC", tag="psC")
                    for k in range(KT):
                        lhsT = wt[:, k * 128:(k + 1) * 128]
                        nc.tensor.matmul(p[:, 0:512], lhsT,
                                         hT_full[k][:, 0:512],
                                         start=(k == 0), stop=(k == KT - 1))
                        nc.tensor.matmul(p[:, 512:1024], lhsT,
                                         hT_full[k][:, 512:1024],
                                         start=(k == 0), stop=(k == KT - 1))
                    lo = lop.tile([128, B], dt.float32, name="lo", tag="lo")
                    nc.scalar.activation(lo[:], p[:], TANH,
                                         bias=bout_t[:, v:v + 1])
                    nc.sync.dma_start(logit_sh[v * 128:(v + 1) * 128, :],
                                      lo[:])

    nc.compile()
    return nc


def kernel(city_input_idx, hidden_state, W_ih, b_ih, W_hh, b_hh, W_out, b_out):
    global LAST_RESULT
    idx = np.asarray(city_input_idx).astype(np.int64)
    h = np.asarray(hidden_state, dtype=np.float32)[0]          # [B, H]
    W_ih = np.asarray(W_ih, dtype=np.float32)                  # [3H, V]
    b_ih = np.asarray(b_ih, dtype=np.float32)
    W_hh = np.asarray(W_hh, dtype=np.float32)                  # [3H, H]
    b_hh = np.asarray(b_hh, dtype=np.float32)
    W_out = np.asarray(W_out, dtype=np.float32)                # [V, H]
    b_out = np.asarray(b_out, dtype=np.float32)

    # ---- host-side sharding / layout ----
    owner = (idx // VS).astype(np.int64)
    perm = np.argsort(owner, kind="stable")
    cnts = np.bincount(owner, minlength=N_CORES).astype(np.int64)
    offs = np.zeros(N_CORES + 1, dtype=np.int64)
    offs[1:] = np.cumsum(cnts)
    c_tiles = max(2, int(-(-int(cnts.max()) // 128)))
    C = 128 * c_tiles

    loc_sorted = (idx[perm] - owner[perm] * VS).astype(np.int32)
    h_perm = h[perm]

    # replicated weights
    whhT_np = np.zeros((KA * 128, G), dtype=np.float32)
    whhT_np[:H] = W_hh.T
    whhT_np[H] = b_hh

    key = (c_tiles, tuple(cnts.tolist()))
    nc = _BUILD_CACHE.get(key)
    if nc is None:
        nc = _build(c_tiles, cnts, offs)
        _BUILD_CACHE[key] = nc

    in_maps = []
    for k in range(N_CORES):
        cnt = int(cnts[k])
        off = int(offs[k])

        tab = W_ih[:, k * VS:(k + 1) * VS].T + b_ih[None, :]   # [VS, 3H]

        li = np.zeros((C,), dtype=np.int32)
        li[:cnt] = loc_sorted[off:off + cnt]

        hs = np.zeros((C, H), dtype=np.float32)
        hs[:cnt] = h_perm[off:off + cnt]

        hsT = np.zeros((KA * 128, C), dtype=np.float32)
        hsT[:H, :cnt] = hs[:cnt].T
        hsT[H] = 1.0

        wsh = W_out[k * VS:(k + 1) * VS]                       # [VS, H]
        wsp = np.zeros((VP, H), dtype=np.float32)
        wsp[:VS] = wsh
        # wout[v, i, kt*128+j] = wsp[v*128+j, kt*128+i]
        wup = np.ascontiguousarray(
            wsp.reshape(NV, 128, KT, 128).transpose(0, 3, 2, 1)
            .reshape(NV, 128, H))

        bo = np.zeros((VP,), dtype=np.float32)
        bo[:VS] = b_out[k * VS:(k + 1) * VS]
        bo = np.ascontiguousarray(bo.reshape(NV, 128).T)       # [128, NV]

        in_maps.append({
            "table": np.ascontiguousarray(tab),
            "locidx": li.reshape(c_tiles, 128, 1),
            "hsegT": hsT,
            "hseg": hs,
            "whhT": whhT_np,
            "wout": wup,
            "bout": bo,
        })

    res = bass_utils.run_bass_kernel_spmd(
        nc, in_maps, core_ids=list(range(N_CORES)), trace=TRACE)
    LAST_RESULT = res

    # ---- host-side unshard ----
    logit_vb = np.concatenate(
        [res.results[k]["logit_sh"][:VS] for k in range(N_CORES)], axis=0)
    logit = np.empty((B, V), dtype=np.float32)
    logit[perm, :] = logit_vb.T

    hn_perm = np.concatenate(
        [res.results[k]["hseg_out"][:int(cnts[k])] for k in range(N_CORES)],
        axis=0)
    h_new = np.empty((B, H), dtype=np.float32)
    h_new[perm] = hn_perm

    return logit, h_new[None], h_new


# revision 10
# speedup vs baseline: 1.1603x; 1.1603x over previous
"""CityGRU (embedding-lookup GRU cell + vocab projection) on 8 TRN2 NeuronCores.

Reference computation (B=1024, H=1024, vocab=50000):
    x_gates = W_ih.T[idx] + b_ih                  # embedding gather  [B, 3H]
    h_gates = h @ W_hh.T + b_hh                   # [B, 3H]
    r = sigmoid(xr + hr); z = sigmoid(xz + hz); n = tanh(xn + r * hn)
    h_new = (1 - z) * n + z * h
    logit = tanh(h_new @ W_out.T + b_out)         # [B, 50000]

Sharding (8 cores, tensor-parallel over the vocab):
  * core k owns vocab rows [k*6250, (k+1)*6250) of W_ih / W_out / b_out.
  * the batch is host-permuted so that core k's GRU rows are exactly the
    indices its W_ih shard owns; the gather happens on-device via
    indirect DMA from the core's local (bias-folded) embedding table.
  * each core computes the GRU for its ~128 rows (padded to a uniform
    capacity), transposes h_new_seg, AllGathers the transposed segments,
    compacts them into the full h_newT [H, B], then computes its vocab
    shard of the logit matmul + tanh.
  * matmuls run in float32r (full-rate fp32, ~1e-4 rel err).

Host does only sharding/layout work: permutation, padding, weight
transposes, bias folding, and final unpermute/concat.
"""

import ml_dtypes
import numpy as np

import concourse.bass as bass
import concourse.mybir as mybir
import concourse.tile as tile
from concourse import bacc
from concourse import bass_utils
from concourse.masks import make_identity

N_CORES = 8
B = 1024
H = 1024
G = 3 * H          # 3072 gate width
V = 50000
VS = V // N_CORES  # 6250 vocab rows per core
NV = 49            # vocab tiles of 128 per core (6272 padded)
VP = NV * 128      # 6272
KT = H // 128      # 8 hidden k-tiles
KA = KT + 1        # 9 augmented k-tiles (bias row)

# Set by an external harness to request NTFF tracing (requires the
# antenv.axon_hooks profiling hook to be installed by that harness).
TRACE = False
LAST_RESULT = None  # BassKernelResults of the most recent run (for harnesses)

_BUILD_CACHE = {}


def _build(c_tiles, cnts, offs):
    """Build + compile the SPMD Bass program.

    c_tiles: number of 128-row tiles of per-core GRU batch capacity.
    cnts/offs: per-rank segment sizes/offsets in the permuted batch
    (baked into the compaction DMA addressing; identical on all cores).
    """
    dt = mybir.dt
    C = 128 * c_tiles

    nc = bacc.Bacc("TRN2", target_bir_lowering=False, debug=False,
                   num_devices=N_CORES)

    # ---- per-core inputs ----
    table = nc.dram_tensor("table", [VS, G], dt.float32, kind="ExternalInput")
    locidx = nc.dram_tensor("locidx", [c_tiles, 128, 1], dt.int32,
                            kind="ExternalInput")
    hsegT = nc.dram_tensor("hsegT", [KA * 128, C], dt.float32r,
                           kind="ExternalInput")
    hseg = nc.dram_tensor("hseg", [C, H], dt.float32, kind="ExternalInput")
    whhT = nc.dram_tensor("whhT", [KA * 128, G], dt.float32r,
                          kind="ExternalInput")
    wout = nc.dram_tensor("wout", [NV, 128, H], dt.bfloat16,
                          kind="ExternalInput")
    bout = nc.dram_tensor("bout", [128, NV], dt.float32, kind="ExternalInput")

    # ---- per-core outputs ----
    logit_sh = nc.dram_tensor("logit_sh", [VP, B], dt.float32,
                              kind="ExternalOutput")
    hseg_out = nc.dram_tensor("hseg_out", [C, H], dt.float32,
                              kind="ExternalOutput")

    SIG = mybir.ActivationFunctionType.Sigmoid
    TANH = mybir.ActivationFunctionType.Tanh

    with tile.TileContext(nc) as tc:
        with (
            tc.tile_pool(name="pers", bufs=1) as pers,
            tc.tile_pool(name="dram", bufs=1, space="DRAM") as dram,
        ):
            ident = pers.tile([128, 128], dt.float32, name="ident", tag="ident")
            make_identity(nc, ident[:])

            bout_t = pers.tile([128, NV], dt.float32, name="bout", tag="bout")
            nc.sync.dma_start(bout_t[:], bout[:])

            # h_newT (full, [H, B]: k-tile k in cols [k*B, (k+1)*B)) --
            # filled from the AllGather result
            hT_big = pers.tile([128, KT * B], dt.bfloat16, name="hTbig",
                               tag="hTbig")

            ag_in = dram.tile([KT * 128, C], dt.bfloat16)
            ag_out = dram.tile([N_CORES * KT * 128, C], dt.bfloat16)

            # ---------------- Phase A: gather + GRU on the local segment ----
            x_seg = []
            hseg_t = []
            hnew_t = []
            for mt in range(c_tiles):
                idx_t = pers.tile([128, 1], dt.int32, name=f"idx{mt}", tag=f"idx{mt}")
                nc.sync.dma_start(idx_t[:], locidx[mt])
                xs = pers.tile([128, G], dt.float32, name=f"xseg{mt}", tag=f"xseg{mt}")
                nc.gpsimd.indirect_dma_start(
                    out=xs[:],
                    out_offset=None,
                    in_=table[:],
                    in_offset=bass.IndirectOffsetOnAxis(ap=idx_t[:, :1], axis=0),
                )
                x_seg.append(xs)
                hs = pers.tile([128, H], dt.float32, name=f"hseg{mt}", tag=f"hseg{mt}")
                nc.sync.dma_start(hs[:], hseg[mt * 128:(mt + 1) * 128, :])
                hseg_t.append(hs)
                hnew_t.append(pers.tile([128, H], dt.float32, name=f"hnew{mt}", tag=f"hnew{mt}"))

            hsegT_t = []
            for k in range(KA):
                t = pers.tile([128, C], dt.float32r, name=f"hsT{k}", tag=f"hsT{k}")
                nc.sync.dma_start(t[:], hsegT[k * 128:(k + 1) * 128, :])
                hsegT_t.append(t)

            with (
                tc.tile_pool(name="whhp", bufs=12) as whhp,
                tc.tile_pool(name="psA", bufs=6, space="PSUM") as psA,
                tc.tile_pool(name="gate", bufs=1) as gate,
            ):
                r_t = [gate.tile([128, H], dt.float32, name=f"r{mt}", tag=f"r{mt}")
                       for mt in range(c_tiles)]
                z_t = [gate.tile([128, H], dt.float32, name=f"z{mt}", tag=f"z{mt}")
                       for mt in range(c_tiles)]
                n_t = [gate.tile([128, H], dt.float32, name=f"n{mt}", tag=f"n{mt}")
                       for mt in range(c_tiles)]
                tmp_r = [gate.tile([128, H], dt.float32, name=f"tr{mt}", tag=f"tr{mt}")
                         for mt in range(c_tiles)]
                tmp_z = [gate.tile([128, H], dt.float32, name=f"tz{mt}", tag=f"tz{mt}")
                         for mt in range(c_tiles)]
                tmp_n = [gate.tile([128, H], dt.float32, name=f"tn{mt}", tag=f"tn{mt}")
                         for mt in range(c_tiles)]
                tmp_h = [gate.tile([128, H], dt.float32, name=f"th{mt}", tag=f"th{mt}")
                         for mt in range(c_tiles)]

                # h_gates = hseg @ W_hh_aug.T in 512-wide gate chunks.
                ps = {}
                for n in range(6):
                    w_nk = []
                    for k in range(KA):
                        w = whhp.tile([128, 512], dt.float32r, name="whh", tag="whh")
                        nc.sync.dma_start(
                            w[:], whhT[k * 128:(k + 1) * 128,
                                       n * 512:(n + 1) * 512])
                        w_nk.append(w)
                    for mt in range(c_tiles):
                        p = psA.tile([128, 512], dt.float32, name="psA", tag="psA")
                        for k in range(KA):
                            nc.tensor.matmul(
                                p[:],
                                hsegT_t[k][:, mt * 128:(mt + 1) * 128],
                                w_nk[k][:],
                                start=(k == 0),
                                stop=(k == KA - 1),
                            )
                        ps[(mt, n)] = p

                    # consume psum pairs as soon as a gate's two chunks exist
                    if n == 1:  # r gate: chunks 0,1
                        for mt in range(c_tiles):
                            for j in (0, 1):
                                nc.vector.tensor_add(
                                    tmp_r[mt][:, j * 512:(j + 1) * 512],
                                    ps[(mt, j)][:],
                                    x_seg[mt][:, j * 512:(j + 1) * 512])
                            nc.scalar.activation(r_t[mt][:], tmp_r[mt][:], SIG)
                    if n == 3:  # z gate: chunks 2,3
                        for mt in range(c_tiles):
                            for j in (2, 3):
                                nc.vector.tensor_add(
                                    tmp_z[mt][:, (j - 2) * 512:(j - 1) * 512],
                                    ps[(mt, j)][:],
                                    x_seg[mt][:, j * 512:(j + 1) * 512])
                            nc.scalar.activation(z_t[mt][:], tmp_z[mt][:], SIG)
                    if n == 5:  # n gate: chunks 4,5 (x + r * h_gates)
                        for mt in range(c_tiles):
                            for j in (4, 5):
                                s = slice((j - 4) * 512, (j - 3) * 512)
                                nc.vector.tensor_mul(
                                    tmp_n[mt][:, s], r_t[mt][:, s],
                                    ps[(mt, j)][:])
                                nc.vector.tensor_add(
                                    tmp_n[mt][:, s], tmp_n[mt][:, s],
                                    x_seg[mt][:, j * 512:(j + 1) * 512])
                            nc.scalar.activation(n_t[mt][:], tmp_n[mt][:], TANH)

                # h_new = n + z * (h - n)
                for mt in range(c_tiles):
                    nc.vector.tensor_sub(tmp_h[mt][:], hseg_t[mt][:],
                                         n_t[mt][:])
                    nc.vector.tensor_mul(tmp_h[mt][:], z_t[mt][:],
                                         tmp_h[mt][:])
                    nc.vector.tensor_add(hnew_t[mt][:], n_t[mt][:],
                                         tmp_h[mt][:])
                    nc.sync.dma_start(hseg_out[mt * 128:(mt + 1) * 128, :],
                                      hnew_t[mt][:])

                # transpose h_new_seg -> [H, C] tiles, ship to the AllGather
                hnT = [gate.tile([128, C], dt.bfloat16, name=f"hnT{k}", tag=f"hnT{k}")
                       for k in range(KT)]
                with tc.tile_pool(name="psT", bufs=2, space="PSUM") as psT:
                    for mt in range(c_tiles):
                        for k in range(KT):
                            pt = psT.tile([128, 128], dt.float32, name="psT", tag="psT")
                            nc.tensor.transpose(
                                pt[:],
                                hnew_t[mt][:, k * 128:(k + 1) * 128],
                                ident[:])
                            nc.vector.tensor_copy(
                                hnT[k][:, mt * 128:(mt + 1) * 128], pt[:])
                for k in range(KT):
                    nc.sync.dma_start(ag_in[k * 128:(k + 1) * 128, :],
                                      hnT[k][:])

                # ---------------- Phase B: AllGather + compaction ----------
                nc.gpsimd.collective_compute(
                    "AllGather",
                    mybir.AluOpType.bypass,
                    ins=[ag_in.opt()],
                    outs=[ag_out.opt()],
                    replica_groups=[list(range(N_CORES))],
                )
                # One compaction DMA per rank: ag_out rows
                # [r*KT*128 .. (r+1)*KT*128) viewed as [KT, 128, C] map to
                # partition p, k-tile k, col off+c of the big h_newT tile.
                ag_v = ag_out[:].rearrange("(r k p) c -> r p k c", r=N_CORES,
                                           k=KT, p=128)
                for r in range(N_CORES):
                    cnt = int(cnts[r])
                    if cnt == 0:
                        continue
                    off = int(offs[r])
                    nc.sync.dma_start(
                        hT_big[:].rearrange("p (k b) -> p k b", k=KT)[
                            :, :, off:off + cnt],
                        ag_v[r, :, :, 0:cnt])

            # ---------------- Phase C: logit = tanh(Wout_sh @ h_newT + b) --
            with (
                tc.tile_pool(name="woutp", bufs=8) as woutp,
                tc.tile_pool(name="lop", bufs=4) as lop,
                tc.tile_pool(name="psC", bufs=3, space="PSUM") as psC,
            ):
                for v in range(NV):
                    wt = woutp.tile([128, H], dt.bfloat16, name="wout", tag="wout")
                    nc.sync.dma_start(wt[:], wout[v])
                    p = psC.tile([128, B], dt.float32, name="psC", tag="psC")
                    for k in range(KT):
                        lhsT = wt[:, k * 128:(k + 1) * 128]
                        nc.tensor.matmul(p[:, 0:512], lhsT,
                                         hT_big[:, k * B:k * B + 512],
                                         start=(k == 0), stop=(k == KT - 1))
                        nc.tensor.matmul(p[:, 512:1024], lhsT,
                                         hT_big[:, k * B + 512:(k + 1) * B],
                                         start=(k == 0), stop=(k == KT - 1))
                    lo = lop.tile([128, B], dt.float32, name="lo", tag="lo")
                    nc.scalar.activation(lo[:], p[:], TANH,
                                         bias=bout_t[:, v:v + 1])
                    nc.sync.dma_start(logit_sh[v * 128:(v + 1) * 128, :],
                                      lo[:])

    nc.compile()
    return nc


def kernel(city_input_idx, hidden_state, W_ih, b_ih, W_hh, b_hh, W_out, b_out):
    global LAST_RESULT
    idx = np.asarray(city_input_idx).astype(np.int64)
    h = np.asarray(hidden_state, dtype=np.float32)[0]          # [B, H]
    W_ih = np.asarray(W_ih, dtype=np.float32)                  # [3H, V]
    b_ih = np.asarray(b_ih, dtype=np.float32)
    W_hh = np.asarray(W_hh, dtype=np.float32)                  # [3H, H]
    b_hh = np.asarray(b_hh, dtype=np.float32)
    W_out = np.asarray(W_out, dtype=np.float32)                # [V, H]
    b_out = np.asarray(b_out, dtype=np.float32)

    # ---- host-side sharding / layout ----
    owner = (idx // VS).astype(np.int64)
    perm = np.argsort(owner, kind="stable")
    cnts = np.bincount(owner, minlength=N_CORES).astype(np.int64)
    offs = np.zeros(N_CORES + 1, dtype=np.int64)
    offs[1:] = np.cumsum(cnts)
    c_tiles = max(2, int(-(-int(cnts.max()) // 128)))
    C = 128 * c_tiles

    loc_sorted = (idx[perm] - owner[perm] * VS).astype(np.int32)
    h_perm = h[perm]

    # replicated weights
    whhT_np = np.zeros((KA * 128, G), dtype=np.float32)
    whhT_np[:H] = W_hh.T
    whhT_np[H] = b_hh

    key = (c_tiles, tuple(cnts.tolist()))
    nc = _BUILD_CACHE.get(key)
    if nc is None:
        nc = _build(c_tiles, cnts, offs)
        _BUILD_CACHE[key] = nc

    in_maps = []
    for k in range(N_CORES):
        cnt = int(cnts[k])
        off = int(offs[k])

        tab = W_ih[:, k * VS:(k + 1) * VS].T + b_ih[None, :]   # [VS, 3H]

        li = np.zeros((C,), dtype=np.int32)
        li[:cnt] = loc_sorted[off:off + cnt]

        hs = np.zeros((C, H), dtype=np.float32)
        hs[:cnt] = h_perm[off:off + cnt]

        hsT = np.zeros((KA * 128, C), dtype=np.float32)
        hsT[:H, :cnt] = hs[:cnt].T
        hsT[H] = 1.0

        wsh = W_out[k * VS:(k + 1) * VS]                       # [VS, H]
        wsp = np.zeros((VP, H), dtype=np.float32)
        wsp[:VS] = wsh
        # wout[v, i, kt*128+j] = wsp[v*128+j, kt*128+i]
        wup = np.ascontiguousarray(
            wsp.reshape(NV, 128, KT, 128).transpose(0, 3, 2, 1)
            .reshape(NV, 128, H).astype(ml_dtypes.bfloat16))

        bo = np.zeros((VP,), dtype=np.float32)
        bo[:VS] = b_out[k * VS:(k + 1) * VS]
        bo = np.ascontiguousarray(bo.reshape(NV, 128).T)       # [128, NV]

        in_maps.append({
            "table": np.ascontiguousarray(tab),
            "locidx": li.reshape(c_tiles, 128, 1),
            "hsegT": hsT,
            "hseg": hs,
            "whhT": whhT_np,
            "wout": wup,
            "bout": bo,
        })

    res = bass_utils.run_bass_kernel_spmd(
        nc, in_maps, core_ids=list(range(N_CORES)), trace=TRACE)
    LAST_RESULT = res

    # ---- host-side unshard ----
    logit_vb = np.concatenate(
        [res.results[k]["logit_sh"][:VS] for k in range(N_CORES)], axis=0)
    logit = np.empty((B, V), dtype=np.float32)
    logit[perm, :] = logit_vb.T

    hn_perm = np.concatenate(
        [res.results[k]["hseg_out"][:int(cnts[k])] for k in range(N_CORES)],
        axis=0)
    h_new = np.empty((B, H), dtype=np.float32)
    h_new[perm] = hn_perm

    return logit, h_new[None], h_new


# revision 12
# speedup vs baseline: 1.1865x; 1.0226x over previous
"""CityGRU (embedding-lookup GRU cell + vocab projection) on 8 TRN2 NeuronCores.

Reference computation (B=1024, H=1024, vocab=50000):
    x_gates = W_ih.T[idx] + b_ih                  # embedding gather  [B, 3H]
    h_gates = h @ W_hh.T + b_hh                   # [B, 3H]
    r = sigmoid(xr + hr); z = sigmoid(xz + hz); n = tanh(xn + r * hn)
    h_new = (1 - z) * n + z * h
    logit = tanh(h_new @ W_out.T + b_out)         # [B, 50000]

Sharding (8 cores, tensor-parallel over the vocab):
  * core k owns vocab rows [k*6250, (k+1)*6250) of W_ih / W_out / b_out.
  * the batch is host-permuted so that core k's GRU rows are exactly the
    indices its W_ih shard owns; the gather happens on-device via
    indirect DMA from the core's local (bias-folded) embedding table.
  * each core computes the GRU for its ~128 rows (padded to a uniform
    capacity), transposes h_new_seg, AllGathers the transposed segments,
    compacts them into the full h_newT [H, B], then computes its vocab
    shard of the logit matmul + tanh.
  * matmuls run in bf16 with fp32 PSUM accumulation (~5e-3 rel err).

Host does only sharding/layout work: permutation, padding, weight
transposes, bias folding, and final unpermute/concat.
"""

import ml_dtypes
import numpy as np

import concourse.bass as bass
import concourse.mybir as mybir
import concourse.tile as tile
from concourse import bacc
from concourse import bass_utils
from concourse.masks import make_identity

N_CORES = 8
B = 1024
H = 1024
G = 3 * H          # 3072 gate width
V = 50000
VS = V // N_CORES  # 6250 vocab rows per core
NV = 49            # vocab tiles of 128 per core (6272 padded)
VP = NV * 128      # 6272
KT = H // 128      # 8 hidden k-tiles
KA = KT + 1        # 9 augmented k-tiles (bias row)

# Set by an external harness to request NTFF tracing (requires the
# antenv.axon_hooks profiling hook to be installed by that harness).
TRACE = False
LAST_RESULT = None  # BassKernelResults of the most recent run (for harnesses)

_BUILD_CACHE = {}


def _build(c_tiles, cnts, offs):
    """Build + compile the SPMD Bass program.

    c_tiles: number of 128-row tiles of per-core GRU batch capacity.
    cnts/offs: per-rank segment sizes/offsets in the permuted batch
    (baked into the compaction DMA addressing; identical on all cores).
    """
    dt = mybir.dt
    C = 128 * c_tiles

    nc = bacc.Bacc("TRN2", target_bir_lowering=False, debug=False,
                   num_devices=N_CORES)

    # ---- per-core inputs ----
    table = nc.dram_tensor("table", [VS, G], dt.float32, kind="ExternalInput")
    locidx = nc.dram_tensor("locidx", [c_tiles, 128, 1], dt.int32,
                            kind="ExternalInput")
    hsegT = nc.dram_tensor("hsegT", [KA * 128, C], dt.bfloat16,
                           kind="ExternalInput")
    hseg = nc.dram_tensor("hseg", [C, H], dt.float32, kind="ExternalInput")
    whhT = nc.dram_tensor("whhT", [KA * 128, G], dt.bfloat16,
                          kind="ExternalInput")
    wout = nc.dram_tensor("wout", [NV, 128, H], dt.bfloat16,
                          kind="ExternalInput")
    bout = nc.dram_tensor("bout", [128, NV], dt.float32, kind="ExternalInput")

    # ---- per-core outputs ----
    logit_sh = nc.dram_tensor("logit_sh", [VP, B], dt.float32,
                              kind="ExternalOutput")
    hseg_out = nc.dram_tensor("hseg_out", [C, H], dt.float32,
                              kind="ExternalOutput")

    SIG = mybir.ActivationFunctionType.Sigmoid
    TANH = mybir.ActivationFunctionType.Tanh

    with tile.TileContext(nc) as tc:
        with (
            tc.tile_pool(name="pers", bufs=1) as pers,
            tc.tile_pool(name="dram", bufs=1, space="DRAM") as dram,
        ):
            ident = pers.tile([128, 128], dt.float32, name="ident", tag="ident")
            make_identity(nc, ident[:])

            bout_t = pers.tile([128, NV], dt.float32, name="bout", tag="bout")
            nc.sync.dma_start(bout_t[:], bout[:])

            # h_newT (full, [H, B]: k-tile k in cols [k*B, (k+1)*B)) --
            # filled from the AllGather result
            hT_big = pers.tile([128, KT * B], dt.bfloat16, name="hTbig",
                               tag="hTbig")

            # AllGather payload is trimmed to the widest real segment
            c_ag = max(16, -(-int(max(cnts)) // 16) * 16)
            ag_in = dram.tile([KT * 128, c_ag], dt.bfloat16)
            ag_out = dram.tile([N_CORES * KT * 128, c_ag], dt.bfloat16)

            # ---------------- Phase A: gather + GRU on the local segment ----
            x_seg = []
            hseg_t = []
            hnew_t = []
            for mt in range(c_tiles):
                idx_t = pers.tile([128, 1], dt.int32, name=f"idx{mt}", tag=f"idx{mt}")
                nc.sync.dma_start(idx_t[:], locidx[mt])
                xs = pers.tile([128, G], dt.float32, name=f"xseg{mt}", tag=f"xseg{mt}")
                nc.gpsimd.indirect_dma_start(
                    out=xs[:],
                    out_offset=None,
                    in_=table[:],
                    in_offset=bass.IndirectOffsetOnAxis(ap=idx_t[:, :1], axis=0),
                )
                x_seg.append(xs)
                hs = pers.tile([128, H], dt.float32, name=f"hseg{mt}", tag=f"hseg{mt}")
                nc.sync.dma_start(hs[:], hseg[mt * 128:(mt + 1) * 128, :])
                hseg_t.append(hs)
                hnew_t.append(pers.tile([128, H], dt.float32, name=f"hnew{mt}", tag=f"hnew{mt}"))

            hsegT_t = []
            for k in range(KA):
                t = pers.tile([128, C], dt.bfloat16, name=f"hsT{k}", tag=f"hsT{k}")
                nc.sync.dma_start(t[:], hsegT[k * 128:(k + 1) * 128, :])
                hsegT_t.append(t)

            with (
                tc.tile_pool(name="whhp", bufs=12) as whhp,
                tc.tile_pool(name="psA", bufs=6, space="PSUM") as psA,
                tc.tile_pool(name="gate", bufs=1) as gate,
            ):
                r_t = [gate.tile([128, H], dt.float32, name=f"r{mt}", tag=f"r{mt}")
                       for mt in range(c_tiles)]
                z_t = [gate.tile([128, H], dt.float32, name=f"z{mt}", tag=f"z{mt}")
                       for mt in range(c_tiles)]
                n_t = [gate.tile([128, H], dt.float32, name=f"n{mt}", tag=f"n{mt}")
                       for mt in range(c_tiles)]
                tmp_r = [gate.tile([128, H], dt.float32, name=f"tr{mt}", tag=f"tr{mt}")
                         for mt in range(c_tiles)]
                tmp_z = [gate.tile([128, H], dt.float32, name=f"tz{mt}", tag=f"tz{mt}")
                         for mt in range(c_tiles)]
                tmp_n = [gate.tile([128, H], dt.float32, name=f"tn{mt}", tag=f"tn{mt}")
                         for mt in range(c_tiles)]
                tmp_h = [gate.tile([128, H], dt.float32, name=f"th{mt}", tag=f"th{mt}")
                         for mt in range(c_tiles)]

                # h_gates = hseg @ W_hh_aug.T in 512-wide gate chunks.
                ps = {}
                for n in range(6):
                    w_nk = []
                    for k in range(KA):
                        w = whhp.tile([128, 512], dt.bfloat16, name="whh", tag="whh")
                        nc.sync.dma_start(
                            w[:], whhT[k * 128:(k + 1) * 128,
                                       n * 512:(n + 1) * 512])
                        w_nk.append(w)
                    for mt in range(c_tiles):
                        p = psA.tile([128, 512], dt.float32, name="psA", tag="psA")
                        for k in range(KA):
                            nc.tensor.matmul(
                                p[:],
                                hsegT_t[k][:, mt * 128:(mt + 1) * 128],
                                w_nk[k][:],
                                start=(k == 0),
                                stop=(k == KA - 1),
                            )
                        ps[(mt, n)] = p

                    # consume psum pairs as soon as a gate's two chunks exist
                    if n == 1:  # r gate: chunks 0,1
                        for mt in range(c_tiles):
                            for j in (0, 1):
                                nc.vector.tensor_add(
                                    tmp_r[mt][:, j * 512:(j + 1) * 512],
                                    ps[(mt, j)][:],
                                    x_seg[mt][:, j * 512:(j + 1) * 512])
                            nc.scalar.activation(r_t[mt][:], tmp_r[mt][:], SIG)
                    if n == 3:  # z gate: chunks 2,3
                        for mt in range(c_tiles):
                            for j in (2, 3):
                                nc.vector.tensor_add(
                                    tmp_z[mt][:, (j - 2) * 512:(j - 1) * 512],
                                    ps[(mt, j)][:],
                                    x_seg[mt][:, j * 512:(j + 1) * 512])
                            nc.scalar.activation(z_t[mt][:], tmp_z[mt][:], SIG)
                    if n == 5:  # n gate: chunks 4,5 (x + r * h_gates)
                        for mt in range(c_tiles):
                            for j in (4, 5):
                                s = slice((j - 4) * 512, (j - 3) * 512)
                                nc.vector.tensor_mul(
                                    tmp_n[mt][:, s], r_t[mt][:, s],
                                    ps[(mt, j)][:])
                                nc.vector.tensor_add(
                                    tmp_n[mt][:, s], tmp_n[mt][:, s],
                                    x_seg[mt][:, j * 512:(j + 1) * 512])
                            nc.scalar.activation(n_t[mt][:], tmp_n[mt][:], TANH)

                # h_new = n + z * (h - n)
                for mt in range(c_tiles):
                    nc.vector.tensor_sub(tmp_h[mt][:], hseg_t[mt][:],
                                         n_t[mt][:])
                    nc.vector.tensor_mul(tmp_h[mt][:], z_t[mt][:],
                                         tmp_h[mt][:])
                    nc.vector.tensor_add(hnew_t[mt][:], n_t[mt][:],
                                         tmp_h[mt][:])
                    nc.sync.dma_start(hseg_out[mt * 128:(mt + 1) * 128, :],
                                      hnew_t[mt][:])

                # transpose h_new_seg -> [H, C] tiles, ship to the AllGather
                hnT = [gate.tile([128, C], dt.bfloat16, name=f"hnT{k}", tag=f"hnT{k}")
                       for k in range(KT)]
                with tc.tile_pool(name="psT", bufs=2, space="PSUM") as psT:
                    for mt in range(c_tiles):
                        for k in range(KT):
                            pt = psT.tile([128, 128], dt.float32, name="psT", tag="psT")
                            nc.tensor.transpose(
                                pt[:],
                                hnew_t[mt][:, k * 128:(k + 1) * 128],
                                ident[:])
                            nc.vector.tensor_copy(
                                hnT[k][:, mt * 128:(mt + 1) * 128], pt[:])
                for k in range(KT):
                    nc.sync.dma_start(ag_in[k * 128:(k + 1) * 128, :],
                                      hnT[k][:, 0:c_ag])

                # ---------------- Phase B: AllGather + compaction ----------
                nc.gpsimd.collective_compute(
                    "AllGather",
                    mybir.AluOpType.bypass,
                    ins=[ag_in.opt()],
                    outs=[ag_out.opt()],
                    replica_groups=[list(range(N_CORES))],
                )
                # One compaction DMA per rank: ag_out rows
                # [r*KT*128 .. (r+1)*KT*128) viewed as [KT, 128, C] map to
                # partition p, k-tile k, col off+c of the big h_newT tile.
                ag_v = ag_out[:].rearrange("(r k p) c -> r p k c", r=N_CORES,
                                           k=KT, p=128)
                for r in range(N_CORES):
                    cnt = int(cnts[r])
                    if cnt == 0:
                        continue
                    off = int(offs[r])
                    nc.sync.dma_start(
                        hT_big[:].rearrange("p (k b) -> p k b", k=KT)[
                            :, :, off:off + cnt],
                        ag_v[r, :, :, 0:cnt])

            # ---------------- Phase C: logit = tanh(Wout_sh @ h_newT + b) --
            with (
                tc.tile_pool(name="woutp", bufs=8) as woutp,
                tc.tile_pool(name="lop", bufs=4) as lop,
                tc.tile_pool(name="psC", bufs=3, space="PSUM") as psC,
            ):
                for v in range(NV):
                    wt = woutp.tile([128, H], dt.bfloat16, name="wout", tag="wout")
                    nc.sync.dma_start(wt[:], wout[v])
                    p = psC.tile([128, B], dt.float32, name="psC", tag="psC")
                    for k in range(KT):
                        lhsT = wt[:, k * 128:(k + 1) * 128]
                        nc.tensor.matmul(p[:, 0:512], lhsT,
                                         hT_big[:, k * B:k * B + 512],
                                         start=(k == 0), stop=(k == KT - 1))
                        nc.tensor.matmul(p[:, 512:1024], lhsT,
                                         hT_big[:, k * B + 512:(k + 1) * B],
                                         start=(k == 0), stop=(k == KT - 1))
                    lo = lop.tile([128, B], dt.float32, name="lo", tag="lo")
                    nc.scalar.activation(lo[:], p[:], TANH,
                                         bias=bout_t[:, v:v + 1])
                    nc.sync.dma_start(logit_sh[v * 128:(v + 1) * 128, :],
                                      lo[:])

    nc.compile()
    return nc


def kernel(city_input_idx, hidden_state, W_ih, b_ih, W_hh, b_hh, W_out, b_out):
    global LAST_RESULT
    idx = np.asarray(city_input_idx).astype(np.int64)
    h = np.asarray(hidden_state, dtype=np.float32)[0]          # [B, H]
    W_ih = np.asarray(W_ih, dtype=np.float32)                  # [3H, V]
    b_ih = np.asarray(b_ih, dtype=np.float32)
    W_hh = np.asarray(W_hh, dtype=np.float32)                  # [3H, H]
    b_hh = np.asarray(b_hh, dtype=np.float32)
    W_out = np.asarray(W_out, dtype=np.float32)                # [V, H]
    b_out = np.asarray(b_out, dtype=np.float32)

    # ---- host-side sharding / layout ----
    owner = (idx // VS).astype(np.int64)
    perm = np.argsort(owner, kind="stable")
    cnts = np.bincount(owner, minlength=N_CORES).astype(np.int64)
    offs = np.zeros(N_CORES + 1, dtype=np.int64)
    offs[1:] = np.cumsum(cnts)
    c_tiles = max(2, int(-(-int(cnts.max()) // 128)))
    C = 128 * c_tiles

    loc_sorted = (idx[perm] - owner[perm] * VS).astype(np.int32)
    h_perm = h[perm]

    # replicated weights
    whhT_np = np.zeros((KA * 128, G), dtype=np.float32)
    whhT_np[:H] = W_hh.T
    whhT_np[H] = b_hh
    whhT_bf = whhT_np.astype(ml_dtypes.bfloat16)

    key = (c_tiles, tuple(cnts.tolist()))
    nc = _BUILD_CACHE.get(key)
    if nc is None:
        nc = _build(c_tiles, cnts, offs)
        _BUILD_CACHE[key] = nc

    in_maps = []
    for k in range(N_CORES):
        cnt = int(cnts[k])
        off = int(offs[k])

        tab = W_ih[:, k * VS:(k + 1) * VS].T + b_ih[None, :]   # [VS, 3H]

        li = np.zeros((C,), dtype=np.int32)
        li[:cnt] = loc_sorted[off:off + cnt]

        hs = np.zeros((C, H), dtype=np.float32)
        hs[:cnt] = h_perm[off:off + cnt]

        hsT = np.zeros((KA * 128, C), dtype=np.float32)
        hsT[:H, :cnt] = hs[:cnt].T
        hsT[H] = 1.0

        wsh = W_out[k * VS:(k + 1) * VS]                       # [VS, H]
        wsp = np.zeros((VP, H), dtype=np.float32)
        wsp[:VS] = wsh
        # wout[v, i, kt*128+j] = wsp[v*128+j, kt*128+i]
        wup = np.ascontiguousarray(
            wsp.reshape(NV, 128, KT, 128).transpose(0, 3, 2, 1)
            .reshape(NV, 128, H).astype(ml_dtypes.bfloat16))

        bo = np.zeros((VP,), dtype=np.float32)
        bo[:VS] = b_out[k * VS:(k + 1) * VS]
        bo = np.ascontiguousarray(bo.reshape(NV, 128).T)       # [128, NV]

        in_maps.append({
            "table": np.ascontiguousarray(tab),
            "locidx": li.reshape(c_tiles, 128, 1),
            "hsegT": hsT.astype(ml_dtypes.bfloat16),
            "hseg": hs,
            "whhT": whhT_bf,
            "wout": wup,
            "bout": bo,
        })

    res = bass_utils.run_bass_kernel_spmd(
        nc, in_maps, core_ids=list(range(N_CORES)), trace=TRACE)
    LAST_RESULT = res

    # ---- host-side unshard ----
    logit_vb = np.concatenate(
        [res.results[k]["logit_sh"][:VS] for k in range(N_CORES)], axis=0)
    logit = np.empty((B, V), dtype=np.float32)
    logit[perm, :] = logit_vb.T

    hn_perm = np.concatenate(
        [res.results[k]["hseg_out"][:int(cnts[k])] for k in range(N_CORES)],
        axis=0)
    h_new = np.empty((B, H), dtype=np.float32)
    h_new[perm] = hn_perm

    return logit, h_new[None], h_new


# revision 16
# speedup vs baseline: 1.1882x; 1.0014x over previous
"""CityGRU (embedding-lookup GRU cell + vocab projection) on 8 TRN2 NeuronCores.

Reference computation (B=1024, H=1024, vocab=50000):
    x_gates = W_ih.T[idx] + b_ih                  # embedding gather  [B, 3H]
    h_gates = h @ W_hh.T + b_hh                   # [B, 3H]
    r = sigmoid(xr + hr); z = sigmoid(xz + hz); n = tanh(xn + r * hn)
    h_new = (1 - z) * n + z * h
    logit = tanh(h_new @ W_out.T + b_out)         # [B, 50000]

Sharding (8 cores, tensor-parallel over the vocab):
  * core k owns vocab rows [k*6250, (k+1)*6250) of W_ih / W_out / b_out.
  * the batch is host-permuted so that core k's GRU rows are exactly the
    indices its W_ih shard owns; the gather happens on-device via
    indirect DMA from the core's local (bias-folded) embedding table.
  * each core computes the GRU for its ~128 rows (padded to a uniform
    capacity), transposes h_new_seg, AllGathers the transposed segments,
    compacts them into the full h_newT [H, B], then computes its vocab
    shard of the logit matmul + tanh.
  * matmuls run in bf16 with fp32 PSUM accumulation (~5e-3 rel err).

Host does only sharding/layout work: permutation, padding, weight
transposes, bias folding, and final unpermute/concat.
"""

import ml_dtypes
import numpy as np

import concourse.bass as bass
import concourse.mybir as mybir
import concourse.tile as tile
from concourse import bacc
from concourse import bass_utils
from concourse.masks import make_identity

N_CORES = 8
B = 1024
H = 1024
G = 3 * H          # 3072 gate width
V = 50000
VS = V // N_CORES  # 6250 vocab rows per core
NV = 49            # vocab tiles of 128 per core (6272 padded)
VP = NV * 128      # 6272
KT = H // 128      # 8 hidden k-tiles
KA = KT + 1        # 9 augmented k-tiles (bias row)

# Set by an external harness to request NTFF tracing (requires the
# antenv.axon_hooks profiling hook to be installed by that harness).
TRACE = False
LAST_RESULT = None  # BassKernelResults of the most recent run (for harnesses)

_BUILD_CACHE = {}


def _build(c_tiles, cnts, offs):
    """Build + compile the SPMD Bass program.

    c_tiles: number of 128-row tiles of per-core GRU batch capacity.
    cnts/offs: per-rank segment sizes/offsets in the permuted batch
    (baked into the compaction DMA addressing; identical on all cores).
    """
    dt = mybir.dt
    C = 128 * c_tiles

    nc = bacc.Bacc("TRN2", target_bir_lowering=False, debug=False,
                   num_devices=N_CORES)

    # ---- per-core inputs ----
    table = nc.dram_tensor("table", [VS, G], dt.float32, kind="ExternalInput")
    locidx = nc.dram_tensor("locidx", [c_tiles, 128, 1], dt.int32,
                            kind="ExternalInput")
    hsegT = nc.dram_tensor("hsegT", [KA * 128, C], dt.bfloat16,
                           kind="ExternalInput")
    hseg = nc.dram_tensor("hseg", [C, H], dt.float32, kind="ExternalInput")
    whhT = nc.dram_tensor("whhT", [KA * 128, G], dt.bfloat16,
                          kind="ExternalInput")
    wout = nc.dram_tensor("wout", [NV, 128, H], dt.bfloat16,
                          kind="ExternalInput")
    bout = nc.dram_tensor("bout", [128, NV], dt.float32, kind="ExternalInput")

    # ---- per-core outputs ----
    logit_sh = nc.dram_tensor("logit_sh", [VP, B], dt.float32,
                              kind="ExternalOutput")
    hseg_out = nc.dram_tensor("hseg_out", [C, H], dt.float32,
                              kind="ExternalOutput")

    SIG = mybir.ActivationFunctionType.Sigmoid
    TANH = mybir.ActivationFunctionType.Tanh

    with tile.TileContext(nc) as tc:
        with (
            tc.tile_pool(name="pers", bufs=1) as pers,
            tc.tile_pool(name="dram", bufs=1, space="DRAM") as dram,
        ):
            ident = pers.tile([128, 128], dt.float32, name="ident", tag="ident")
            make_identity(nc, ident[:])

            bout_t = pers.tile([128, NV], dt.float32, name="bout", tag="bout")
            nc.sync.dma_start(bout_t[:], bout[:])

            # h_newT (full, [H, B]: k-tile k in cols [k*B, (k+1)*B)) --
            # filled from the AllGather result
            hT_big = pers.tile([128, KT * B], dt.bfloat16, name="hTbig",
                               tag="hTbig")

            # AllGather payload is trimmed to the widest real segment
            c_ag = max(16, -(-int(max(cnts)) // 16) * 16)
            ag_in = dram.tile([KT * 128, c_ag], dt.bfloat16)
            ag_out = dram.tile([N_CORES * KT * 128, c_ag], dt.bfloat16)

            # ---------------- Phase A: gather + GRU on the local segment ----
            x_seg = []
            hseg_t = []
            hnew_t = []
            for mt in range(c_tiles):
                idx_t = pers.tile([128, 1], dt.int32, name=f"idx{mt}", tag=f"idx{mt}")
                nc.sync.dma_start(idx_t[:], locidx[mt])
                xs = pers.tile([128, G], dt.float32, name=f"xseg{mt}", tag=f"xseg{mt}")
                nc.gpsimd.indirect_dma_start(
                    out=xs[:],
                    out_offset=None,
                    in_=table[:],
                    in_offset=bass.IndirectOffsetOnAxis(ap=idx_t[:, :1], axis=0),
                )
                x_seg.append(xs)
                hs = pers.tile([128, H], dt.float32, name=f"hseg{mt}", tag=f"hseg{mt}")
                nc.sync.dma_start(hs[:], hseg[mt * 128:(mt + 1) * 128, :])
                hseg_t.append(hs)
                hnew_t.append(pers.tile([128, H], dt.float32, name=f"hnew{mt}", tag=f"hnew{mt}"))

            # all KA k-tiles of hsegT in one tile / one DMA:
            # hsT_big[:, k*C + c] = hsegT[k*128 + p, c]
            hsT_big = pers.tile([128, KA * C], dt.bfloat16, name="hsTbig",
                                tag="hsTbig")
            nc.sync.dma_start(
                hsT_big[:].rearrange("p (k c) -> p k c", k=KA),
                hsegT[:].rearrange("(k p) c -> p k c", k=KA, p=128))

            def hsT(k, mt):
                return hsT_big[:, k * C + mt * 128:k * C + (mt + 1) * 128]

            with (
                tc.tile_pool(name="whhp", bufs=1) as whhp,
                tc.tile_pool(name="psA", bufs=6, space="PSUM") as psA,
                tc.tile_pool(name="gate", bufs=1) as gate,
            ):
                # W_hh_aug.T resident: 9 k-tiles of [128, G] bf16 (54 KB/part)
                whh_t = []
                for k in range(KA):
                    w = whhp.tile([128, G], dt.bfloat16, name=f"whh{k}",
                                  tag=f"whh{k}")
                    nc.sync.dma_start(w[:], whhT[k * 128:(k + 1) * 128, :])
                    whh_t.append(w)
                r_t = [gate.tile([128, H], dt.float32, name=f"r{mt}", tag=f"r{mt}")
                       for mt in range(c_tiles)]
                z_t = [gate.tile([128, H], dt.float32, name=f"z{mt}", tag=f"z{mt}")
                       for mt in range(c_tiles)]
                n_t = [gate.tile([128, H], dt.float32, name=f"n{mt}", tag=f"n{mt}")
                       for mt in range(c_tiles)]
                tmp_r = [gate.tile([128, H], dt.float32, name=f"tr{mt}", tag=f"tr{mt}")
                         for mt in range(c_tiles)]
                tmp_z = [gate.tile([128, H], dt.float32, name=f"tz{mt}", tag=f"tz{mt}")
                         for mt in range(c_tiles)]
                tmp_n = [gate.tile([128, H], dt.float32, name=f"tn{mt}", tag=f"tn{mt}")
                         for mt in range(c_tiles)]
                tmp_h = [gate.tile([128, H], dt.float32, name=f"th{mt}", tag=f"th{mt}")
                         for mt in range(c_tiles)]

                # h_gates = hseg @ W_hh_aug.T in 512-wide gate chunks.
                ps = {}
                for n in range(6):
                    for mt in range(c_tiles):
                        p = psA.tile([128, 512], dt.float32, name="psA", tag="psA")
                        for k in range(KA):
                            nc.tensor.matmul(
                                p[:],
                                hsT(k, mt),
                                whh_t[k][:, n * 512:(n + 1) * 512],
                                start=(k == 0),
                                stop=(k == KA - 1),
                            )
                        ps[(mt, n)] = p

                    # consume psum pairs as soon as a gate's two chunks exist
                    if n == 1:  # r gate: chunks 0,1
                        for mt in range(c_tiles):
                            for j in (0, 1):
                                nc.vector.tensor_add(
                                    tmp_r[mt][:, j * 512:(j + 1) * 512],
                                    ps[(mt, j)][:],
                                    x_seg[mt][:, j * 512:(j + 1) * 512])
                            nc.scalar.activation(r_t[mt][:], tmp_r[mt][:], SIG)
                    if n == 3:  # z gate: chunks 2,3
                        for mt in range(c_tiles):
                            for j in (2, 3):
                                nc.vector.tensor_add(
                                    tmp_z[mt][:, (j - 2) * 512:(j - 1) * 512],
                                    ps[(mt, j)][:],
                                    x_seg[mt][:, j * 512:(j + 1) * 512])
                            nc.scalar.activation(z_t[mt][:], tmp_z[mt][:], SIG)
                    if n == 5:  # n gate: chunks 4,5 (x + r * h_gates)
                        for mt in range(c_tiles):
                            for j in (4, 5):
                                s = slice((j - 4) * 512, (j - 3) * 512)
                                nc.vector.tensor_mul(
                                    tmp_n[mt][:, s], r_t[mt][:, s],
                                    ps[(mt, j)][:])
                                nc.vector.tensor_add(
                                    tmp_n[mt][:, s], tmp_n[mt][:, s],
                                    x_seg[mt][:, j * 512:(j + 1) * 512])
                            nc.scalar.activation(n_t[mt][:], tmp_n[mt][:], TANH)

                # h_new = n + z * (h - n)
                for mt in range(c_tiles):
                    nc.vector.tensor_sub(tmp_h[mt][:], hseg_t[mt][:],
                                         n_t[mt][:])
                    nc.vector.tensor_mul(tmp_h[mt][:], z_t[mt][:],
                                         tmp_h[mt][:])
                    nc.vector.tensor_add(hnew_t[mt][:], n_t[mt][:],
                                         tmp_h[mt][:])
                    nc.sync.dma_start(hseg_out[mt * 128:(mt + 1) * 128, :],
                                      hnew_t[mt][:])

                # transpose h_new_seg -> [H, C], ship to the AllGather
                hnT_big = gate.tile([128, KT * C], dt.bfloat16, name="hnTbig",
                                    tag="hnTbig")
                with tc.tile_pool(name="psT", bufs=2, space="PSUM") as psT:
                    for mt in range(c_tiles):
                        for k in range(KT):
                            pt = psT.tile([128, 128], dt.float32, name="psT", tag="psT")
                            nc.tensor.transpose(
                                pt[:],
                                hnew_t[mt][:, k * 128:(k + 1) * 128],
                                ident[:])
                            nc.vector.tensor_copy(
                                hnT_big[:, k * C + mt * 128:
                                        k * C + (mt + 1) * 128], pt[:])
                nc.sync.dma_start(
                    ag_in[:].rearrange("(k p) c -> p k c", k=KT, p=128),
                    hnT_big[:].rearrange("p (k c) -> p k c", k=KT)[
                        :, :, 0:c_ag])

                # ---------------- Phase B: AllGather + compaction ----------
                nc.gpsimd.collective_compute(
                    "AllGather",
                    mybir.AluOpType.bypass,
                    ins=[ag_in.opt()],
                    outs=[ag_out.opt()],
                    replica_groups=[list(range(N_CORES))],
                )
                # One compaction DMA per rank: ag_out rows
                # [r*KT*128 .. (r+1)*KT*128) viewed as [KT, 128, C] map to
                # partition p, k-tile k, col off+c of the big h_newT tile.
                ag_v = ag_out[:].rearrange("(r k p) c -> r p k c", r=N_CORES,
                                           k=KT, p=128)
                for r in range(N_CORES):
                    cnt = int(cnts[r])
                    if cnt == 0:
                        continue
                    off = int(offs[r])
                    nc.sync.dma_start(
                        hT_big[:].rearrange("p (k b) -> p k b", k=KT)[
                            :, :, off:off + cnt],
                        ag_v[r, :, :, 0:cnt])

            # ---------------- Phase C: logit = tanh(Wout_sh @ h_newT + b) --
            WB = 4  # vocab tiles per W_out load
            with (
                tc.tile_pool(name="woutp", bufs=4) as woutp,
                tc.tile_pool(name="lop", bufs=4) as lop,
                tc.tile_pool(name="psC", bufs=3, space="PSUM") as psC,
            ):
                for vb in range(0, NV, WB):
                    nw = min(WB, NV - vb)
                    wt = woutp.tile([128, WB * H], dt.bfloat16, name="wout",
                                    tag="wout")
                    nc.sync.dma_start(
                        wt[:].rearrange("p (w h) -> p w h", w=WB)[:, 0:nw, :],
                        wout[vb:vb + nw].rearrange("w p h -> p w h"))
                    for w in range(nw):
                        p = psC.tile([128, B], dt.float32, name="psC",
                                     tag="psC")
                        for k in range(KT):
                            lhsT = wt[:, w * H + k * 128:w * H + (k + 1) * 128]
                            nc.tensor.matmul(
                                p[:, 0:512], lhsT,
                                hT_big[:, k * B:k * B + 512],
                                start=(k == 0), stop=(k == KT - 1))
                            nc.tensor.matmul(
                                p[:, 512:1024], lhsT,
                                hT_big[:, k * B + 512:(k + 1) * B],
                                start=(k == 0), stop=(k == KT - 1))
                        lo = lop.tile([128, B], dt.float32, name="lo",
                                      tag="lo")
                        nc.scalar.activation(lo[:], p[:], TANH,
                                             bias=bout_t[:, vb + w:vb + w + 1])
                        nc.scalar.dma_start(
                            logit_sh[(vb + w) * 128:(vb + w + 1) * 128, :],
                            lo[:])

    nc.compile()
    return nc


def kernel(city_input_idx, hidden_state, W_ih, b_ih, W_hh, b_hh, W_out, b_out):
    global LAST_RESULT
    idx = np.asarray(city_input_idx).astype(np.int64)
    h = np.asarray(hidden_state, dtype=np.float32)[0]          # [B, H]
    W_ih = np.asarray(W_ih, dtype=np.float32)                  # [3H, V]
    b_ih = np.asarray(b_ih, dtype=np.float32)
    W_hh = np.asarray(W_hh, dtype=np.float32)                  # [3H, H]
    b_hh = np.asarray(b_hh, dtype=np.float32)
    W_out = np.asarray(W_out, dtype=np.float32)                # [V, H]
    b_out = np.asarray(b_out, dtype=np.float32)

    # ---- host-side sharding / layout ----
    owner = (idx // VS).astype(np.int64)
    perm = np.argsort(owner, kind="stable")
    cnts = np.bincount(owner, minlength=N_CORES).astype(np.int64)
    offs = np.zeros(N_CORES + 1, dtype=np.int64)
    offs[1:] = np.cumsum(cnts)
    c_tiles = max(2, int(-(-int(cnts.max()) // 128)))
    C = 128 * c_tiles

    loc_sorted = (idx[perm] - owner[perm] * VS).astype(np.int32)
    h_perm = h[perm]

    # replicated weights
    whhT_np = np.zeros((KA * 128, G), dtype=np.float32)
    whhT_np[:H] = W_hh.T
    whhT_np[H] = b_hh
    whhT_bf = whhT_np.astype(ml_dtypes.bfloat16)

    key = (c_tiles, tuple(cnts.tolist()))
    nc = _BUILD_CACHE.get(key)
    if nc is None:
        nc = _build(c_tiles, cnts, offs)
        _BUILD_CACHE[key] = nc

    in_maps = []
    for k in range(N_CORES):
        cnt = int(cnts[k])
        off = int(offs[k])

        tab = W_ih[:, k * VS:(k + 1) * VS].T + b_ih[None, :]   # [VS, 3H]

        li = np.zeros((C,), dtype=np.int32)
        li[:cnt] = loc_sorted[off:off + cnt]

        hs = np.zeros((C, H), dtype=np.float32)
        hs[:cnt] = h_perm[off:off + cnt]

        hsT = np.zeros((KA * 128, C), dtype=np.float32)
        hsT[:H, :cnt] = hs[:cnt].T
        hsT[H] = 1.0

        wsh = W_out[k * VS:(k + 1) * VS]                       # [VS, H]
        wsp = np.zeros((VP, H), dtype=np.float32)
        wsp[:VS] = wsh
        # wout[v, i, kt*128+j] = wsp[v*128+j, kt*128+i]
        wup = np.ascontiguousarray(
            wsp.reshape(NV, 128, KT, 128).transpose(0, 3, 2, 1)
            .reshape(NV, 128, H).astype(ml_dtypes.bfloat16))

        bo = np.zeros((VP,), dtype=np.float32)
        bo[:VS] = b_out[k * VS:(k + 1) * VS]
        bo = np.ascontiguousarray(bo.reshape(NV, 128).T)       # [128, NV]

        in_maps.append({
            "table": np.ascontiguousarray(tab),
            "locidx": li.reshape(c_tiles, 128, 1),
            "hsegT": hsT.astype(ml_dtypes.bfloat16),
            "hseg": hs,
            "whhT": whhT_bf,
            "wout": wup,
            "bout": bo,
        })

    res = bass_utils.run_bass_kernel_spmd(
        nc, in_maps, core_ids=list(range(N_CORES)), trace=TRACE)
    LAST_RESULT = res

    # ---- host-side unshard ----
    logit_vb = np.concatenate(
        [res.results[k]["logit_sh"][:VS] for k in range(N_CORES)], axis=0)
    logit = np.empty((B, V), dtype=np.float32)
    logit[perm, :] = logit_vb.T

    hn_perm = np.concatenate(
        [res.results[k]["hseg_out"][:int(cnts[k])] for k in range(N_CORES)],
        axis=0)
    h_new = np.empty((B, H), dtype=np.float32)
    h_new[perm] = hn_perm

    return logit, h_new[None], h_new
